# revision 7
# baseline (speedup 1.0000x reference)
"""Causal self-attention Trainium2 kernel (B=4, T=2048, D=1024, H=16).

Sharding: 8 cores = DP(4 batches) x TP(2 head-groups of 8 heads).
Each core computes, for its (batch b, head-group g):
  qkv projection (f32r matmuls), transposed scores S^T = K Q^T (bf16),
  exp via ScalarE (scale fused), causal mask by triangle-mask multiply,
  A@V with [V|1]-augmented stationary operand (denominator lands in
  PSUM row 64), reciprocal + K=1 broadcast matmul for the division,
  and the output projection (f32r) producing outT = (attn @ Wo_g)^T.
Host sums the two head-group partials per batch and transposes back.
"""

import numpy as np
from contextlib import ExitStack

P = 128
D = 1024
HD = 64
HPC = 8          # heads per core
GF = HPC * HD    # per-core q/k/v feature count = 512
NCHUNK = 512     # token chunk (matmul moving dim)
SCALE = float(D) ** -0.5

_CACHE = {}


def _build(T=2048):
    import concourse.bass as bass  # noqa: F401
    import concourse.bacc as bacc
    import concourse.tile as tile
    from concourse import mybir

    f32 = mybir.dt.float32
    f32r = mybir.dt.float32r
    bf16 = mybir.dt.bfloat16
    Exp = mybir.ActivationFunctionType.Exp
    mult = mybir.AluOpType.mult

    NT = T // NCHUNK          # token chunks (4)
    KT = T // P               # key tiles (16)
    DK = D // P               # contraction tiles over D (8)
    MT = GF // P              # q (and k) feature tiles per core (4)

    nc = bacc.Bacc("TRN2", target_bir_lowering=False, debug=False, num_devices=8)

    xT = nc.dram_tensor("xT", [D, T], f32r, kind="ExternalInput")
    wq = nc.dram_tensor("wq", [D, GF], f32r, kind="ExternalInput")
    wk = nc.dram_tensor("wk", [D, GF], f32r, kind="ExternalInput")
    wv = nc.dram_tensor("wv", [D, GF], f32r, kind="ExternalInput")
    wo = nc.dram_tensor("wo", [GF, D], f32r, kind="ExternalInput")
    outT = nc.dram_tensor("outT", [D, T], f32, kind="ExternalOutput")

    tri_np = np.triu(np.ones((P, P), dtype=np.float32))  # [p, t] = 1 if t >= p
    tri_dram = nc.inline_tensor(tri_np, name="tri_const")

    with tile.TileContext(nc) as tc, ExitStack() as ctx:
        sb = ctx.enter_context(tc.tile_pool(name="sb", bufs=1))
        psum = ctx.enter_context(tc.tile_pool(name="ps", bufs=1, space="PSUM"))

        # ---- constants into SBUF
        tri_sb = sb.tile([P, P], f32, name="tri_sb")
        nc.sync.dma_start(out=tri_sb[:, :], in_=tri_dram.ap())
        zl_sb = sb.tile([P, NCHUNK], f32, name="zl_sb")
        nc.vector.memset(zl_sb[:, :], 0.0)
        on_sb = sb.tile([P, HD], f32, name="on_sb")
        nc.vector.memset(on_sb[:, :], 1.0)
        ones_sb = sb.tile([P, HD], f32r, name="ones_sb")
        nc.vector.tensor_copy(out=ones_sb[64:65, :], in_=on_sb[64:65, :])

        # ---- persistent activations
        # qT/kT: [GF, T] as MT tiles of [128, T], bf16 (head h -> tile h//2,
        # partition offset (h%2)*64)
        q_sb = [sb.tile([P, T], bf16, name=f"q_sb{m}") for m in range(MT)]
        k_sb = [sb.tile([P, T], bf16, name=f"k_sb{m}") for m in range(MT)]
        # v: natural layout, KT tiles of [128 tokens, 8*65] f32 with a ones
        # column appended per head (col h*65+64)
        VW = HPC * (HD + 1)
        v_sb = [sb.tile([P, VW], f32r, name=f"v_sb{t}") for t in range(KT)]
        # attention output (transposed): [GF, T] as MT tiles of [128, T] f32
        a_sb = [sb.tile([P, T], f32r, name=f"a_sb{m}") for m in range(MT)]

        # =========== Phase A1: V projection (x-stationary) ===========
        with tc.tile_pool(name="xv", bufs=2 * DK) as xvp, \
             tc.tile_pool(name="wvp", bufs=1) as wvp:
            wv_sb = wvp.tile([P, DK, GF], f32r, name="wv_sb")
            nc.sync.dma_start(
                out=wv_sb[:, :, :],
                in_=wv[:, :].rearrange("(kk p) f -> p kk f", p=P),
            )
            for tt in range(KT):
                xv = []
                for kk in range(DK):
                    t = xvp.tile([P, P], f32r, name="xv_t", tag="xv")
                    nc.sync.dma_start(
                        out=t[:, :],
                        in_=xT[kk * P:(kk + 1) * P, tt * P:(tt + 1) * P],
                    )
                    xv.append(t)
                vps = psum.tile([P, GF], f32, name="vps", tag="a", bufs=2)
                for kk in range(DK):
                    nc.tensor.matmul(
                        vps[:, :],
                        xv[kk][:, :],
                        wv_sb[:, kk, :],
                        start=(kk == 0), stop=(kk == DK - 1),
                    )
                # scatter into [h*65 : h*65+64] per head, then ones columns
                nc.scalar.copy(
                    out=v_sb[tt][:, :].rearrange("p (h w) -> p h w", w=HD + 1)[:, :, 0:HD],
                    in_=vps[:, :].rearrange("p (h w) -> p h w", w=HD),
                )
                nc.vector.tensor_copy(
                    out=v_sb[tt][:, :].rearrange("p (h w) -> p h w", w=HD + 1)[:, :, HD:HD + 1],
                    in_=on_sb[:, 0:HPC].unsqueeze(2),
                )

        # =========== Phase A2: Q/K projection (W-stationary, n-outer) ===========
        with tc.tile_pool(name="xc", bufs=DK) as xcp, \
             tc.tile_pool(name="wqk", bufs=1) as wqkp:
            w_m = []
            for m in range(MT):
                t = wqkp.tile([P, DK, P], f32r, name=f"wq_m{m}")
                nc.sync.dma_start(
                    out=t[:, :, :],
                    in_=wq[:, m * P:(m + 1) * P].rearrange("(kk p) f -> p kk f", p=P),
                )
                w_m.append((t, q_sb[m]))
            for m in range(MT):
                t = wqkp.tile([P, DK, P], f32r, name=f"wk_m{m}")
                nc.sync.dma_start(
                    out=t[:, :, :],
                    in_=wk[:, m * P:(m + 1) * P].rearrange("(kk p) f -> p kk f", p=P),
                )
                w_m.append((t, k_sb[m]))
            for n in range(NT):
                xc = []
                for kk in range(DK):
                    t = xcp.tile([P, NCHUNK], f32r, name="xc_t", tag="xc")
                    nc.sync.dma_start(
                        out=t[:, :],
                        in_=xT[kk * P:(kk + 1) * P, n * NCHUNK:(n + 1) * NCHUNK],
                    )
                    xc.append(t)
                for (wt, dst) in w_m:
                    qps = psum.tile([P, NCHUNK], f32, name="qps", tag="a", bufs=2)
                    for kk in range(DK):
                        nc.tensor.matmul(
                            qps[:, :],
                            wt[:, kk, :],
                            xc[kk][:, :],
                            start=(kk == 0), stop=(kk == DK - 1),
                        )
                    nc.scalar.copy(
                        out=dst[:, n * NCHUNK:(n + 1) * NCHUNK], in_=qps[:, :]
                    )

        # =========== Phase B: attention ===========
        for h in range(HPC):
            ht = h // 2
            hp = (h % 2) * HD
            for j in range(NT):
                n_i = min(4 * j + 4, KT)
                e_tiles = []
                for i in range(n_i):
                    d = i - 4 * j
                    s_ps = psum.tile([P, NCHUNK], f32, name="s_ps", tag="s", bufs=3)
                    nc.tensor.matmul(
                        s_ps[:, :],
                        k_sb[ht][hp:hp + HD, i * P:(i + 1) * P],
                        q_sb[ht][hp:hp + HD, j * NCHUNK:(j + 1) * NCHUNK],
                        start=True, stop=True,
                    )
                    e_t = sb.tile([P, NCHUNK], f32r, name="e_t", tag="e", bufs=8)
                    if d >= 0:
                        lo = P * d
                        if lo > 0:
                            nc.vector.tensor_copy(out=e_t[:, 0:lo], in_=zl_sb[:, 0:lo])
                        nc.scalar.activation(
                            out=e_t[:, lo:NCHUNK], in_=s_ps[:, lo:NCHUNK],
                            func=Exp, scale=SCALE,
                        )
                        nc.vector.tensor_tensor(
                            out=e_t[:, lo:lo + P], in0=e_t[:, lo:lo + P],
                            in1=tri_sb[:, :].bitcast(f32r), op=mult,
                        )
                    else:
                        nc.scalar.activation(
                            out=e_t[:, :], in_=s_ps[:, :], func=Exp, scale=SCALE,
                        )
                    e_tiles.append(e_t)
                av_ps = psum.tile([P, NCHUNK], f32, name="av_ps", tag="av", bufs=2)
                for i in range(n_i):
                    nc.tensor.matmul(
                        av_ps[0:HD + 1, :],
                        v_sb[i][:, h * (HD + 1):(h + 1) * (HD + 1)],
                        e_tiles[i][:, :],
                        start=(i == 0), stop=(i == n_i - 1),
                    )
                nt_sb = sb.tile([P, NCHUNK], f32, name="nt_sb", tag="nt", bufs=3)
                nc.vector.tensor_copy(out=nt_sb[0:HD + 1, :], in_=av_ps[0:HD + 1, :])
                rt_sb = sb.tile([P, NCHUNK], f32r, name="rt_sb", tag="rt", bufs=2)
                with nc.allow_low_precision(reason="f32r reciprocal: FP22 is plenty for softmax denominators"):
                    nc.vector.reciprocal(out=rt_sb[64:65, :], in_=nt_sb[64:65, :])
                r_ps = psum.tile([HD, NCHUNK], f32, name="r_ps", tag="r", bufs=1)
                nc.tensor.matmul(
                    r_ps[:, :],
                    ones_sb[64:65, :],
                    rt_sb[64:65, :],
                    start=True, stop=True,
                )
                if hp == 0:
                    nc.vector.tensor_tensor(
                        out=a_sb[ht][0:HD, j * NCHUNK:(j + 1) * NCHUNK],
                        in0=nt_sb[0:HD, :], in1=r_ps[:, :], op=mult,
                    )
                else:
                    tmp = sb.tile([HD, NCHUNK], f32r, name="tmp_sb", tag="tmp", bufs=2)
                    nc.vector.tensor_tensor(
                        out=tmp[:, :], in0=nt_sb[0:HD, :], in1=r_ps[:, :], op=mult,
                    )
                    nc.sync.dma_start(
                        out=a_sb[ht][hp:hp + HD, j * NCHUNK:(j + 1) * NCHUNK],
                        in_=tmp[:, :],
                    )

        # =========== Phase C: output projection ===========
        with tc.tile_pool(name="wop", bufs=1) as wop:
            wo_sb = [wop.tile([P, D], f32r, name=f"wo_sb{kk}") for kk in range(MT)]
            for kk in range(MT):
                nc.sync.dma_start(out=wo_sb[kk][:, :], in_=wo[kk * P:(kk + 1) * P, :])
            for m in range(D // P):
                for n in range(NT):
                    ops = psum.tile([P, NCHUNK], f32, name="ops", tag="a", bufs=2)
                    for kk in range(MT):
                        nc.tensor.matmul(
                            ops[:, :],
                            wo_sb[kk][:, m * P:(m + 1) * P],
                            a_sb[kk][:, n * NCHUNK:(n + 1) * NCHUNK],
                            start=(kk == 0), stop=(kk == MT - 1),
                        )
                    oc = sb.tile([P, NCHUNK], f32, name="oc_sb", tag="oc", bufs=3)
                    nc.scalar.copy(out=oc[:, :], in_=ops[:, :])
                    nc.sync.dma_start(
                        out=outT[m * P:(m + 1) * P, n * NCHUNK:(n + 1) * NCHUNK],
                        in_=oc[:, :],
                    )

    nc.compile()
    return nc


def _shard_inputs(x, Wqkv, Wo):
    in_maps = []
    for c in range(8):
        b, g = c // 2, c % 2
        lo, hi = GF * g, GF * (g + 1)
        in_maps.append({
            "xT": np.ascontiguousarray(x[b].T),
            "wq": np.ascontiguousarray(Wqkv[:, lo:hi]),
            "wk": np.ascontiguousarray(Wqkv[:, D + lo:D + hi]),
            "wv": np.ascontiguousarray(Wqkv[:, 2 * D + lo:2 * D + hi]),
            "wo": np.ascontiguousarray(Wo[lo:hi, :]),
        })
    return in_maps


def kernel(x, Wqkv, Wo, _trace=False):
    from concourse.bass_utils import run_bass_kernel_spmd

    x = np.asarray(x, dtype=np.float32)
    Wqkv = np.asarray(Wqkv, dtype=np.float32)
    Wo = np.asarray(Wo, dtype=np.float32)
    B, T, _ = x.shape

    key = ("nc", T)
    if key not in _CACHE:
        _CACHE[key] = _build(T)
    nc = _CACHE[key]

    res = run_bass_kernel_spmd(nc, _shard_inputs(x, Wqkv, Wo), list(range(8)),
                               trace=_trace)
    out = np.empty((B, T, D), dtype=np.float32)
    for b in range(B):
        out[b] = (res.results[2 * b]["outT"] + res.results[2 * b + 1]["outT"]).T
    if _trace:
        kernel.last_results = res
    return out


# revision 9
# speedup vs baseline: 1.1270x; 1.1270x over previous
"""Causal self-attention Trainium2 kernel (B=4, T=2048, D=1024, H=16).

Sharding: 8 cores = DP(4 batches) x TP(2 head-groups of 8 heads).
Per-core pipeline (fused j-outer over 512-token chunks):
  A1: V projection (x-stationary, f32r) into [V|1]-augmented tiles.
  Per chunk j: QK projection chunk (f32r, W-stationary) -> bf16 q/k;
  per head-pair: transposed scores S^T = K Q^T with both heads packed
  into one 2-bank PSUM tile (concurrent PE row groups), one exp
  (ScalarE, scale fused) over both halves, causal zero+triangle mask
  multiply, A@V with [V|1] stationary (denominator in PSUM row 64),
  K=1 broadcast matmul of the denominator and a DVE divide;
  then the output projection chunk (f32r) -> outT partial.
Host sums the two head-group partials per batch and transposes back.
"""

import numpy as np
from contextlib import ExitStack

P = 128
D = 1024
HD = 64
HPC = 8          # heads per core
GF = HPC * HD    # per-core q/k/v feature count = 512
NCHUNK = 512     # token chunk (matmul moving dim)
SCALE = float(D) ** -0.5

_CACHE = {}


def _build(T=2048):
    import concourse.bass as bass  # noqa: F401
    import concourse.bacc as bacc
    import concourse.tile as tile
    from concourse import mybir

    f32 = mybir.dt.float32
    f32r = mybir.dt.float32r
    bf16 = mybir.dt.bfloat16
    Exp = mybir.ActivationFunctionType.Exp
    mult = mybir.AluOpType.mult

    NT = T // NCHUNK          # token chunks (4)
    KT = T // P               # key tiles (16)
    DK = D // P               # contraction tiles over D (8)
    MT = GF // P              # q (and k) feature tiles per core (4)
    NPAIR = HPC // 2          # head pairs (4)
    VW = HPC * (HD + 1)

    nc = bacc.Bacc("TRN2", target_bir_lowering=False, debug=False, num_devices=8)

    xT = nc.dram_tensor("xT", [D, T], f32r, kind="ExternalInput")
    wq = nc.dram_tensor("wq", [D, GF], f32r, kind="ExternalInput")
    wk = nc.dram_tensor("wk", [D, GF], f32r, kind="ExternalInput")
    wv = nc.dram_tensor("wv", [D, GF], f32r, kind="ExternalInput")
    wo = nc.dram_tensor("wo", [GF, D], f32r, kind="ExternalInput")
    outT = nc.dram_tensor("outT", [D, T], f32, kind="ExternalOutput")

    # mask master: cols [384-lo : 512] give [zeros(lo) | upper-triangle]
    tri = np.triu(np.ones((P, P), dtype=np.float32))
    mask_np = np.concatenate([np.zeros((P, 3 * P), np.float32), tri], axis=1)
    mask_dram = nc.inline_tensor(mask_np, name="mask_const")

    with tile.TileContext(nc) as tc, ExitStack() as ctx:
        sb = ctx.enter_context(tc.tile_pool(name="sb", bufs=1))
        psum = ctx.enter_context(tc.tile_pool(name="ps", bufs=1, space="PSUM"))

        # ---- constants
        mk_sb = sb.tile([P, 4 * P], f32, name="mk_sb")
        nc.sync.dma_start(out=mk_sb[:, :], in_=mask_dram.ap())
        on_sb = sb.tile([P, HD], f32, name="on_sb")
        nc.vector.memset(on_sb[:, :], 1.0)
        ones_sb = sb.tile([P, HD], f32r, name="ones_sb")
        nc.vector.tensor_copy(out=ones_sb[64:65, :], in_=on_sb[64:65, :])

        # ---- persistent activations
        k_sb = [sb.tile([P, T], bf16, name=f"k_sb{m}") for m in range(MT)]
        v_sb = [sb.tile([P, VW], f32r, name=f"v_sb{t}") for t in range(KT)]

        # =========== Phase A1: V projection (x-stationary) ===========
        with tc.tile_pool(name="xv", bufs=2 * DK) as xvp, \
             tc.tile_pool(name="wvp", bufs=1) as wvp:
            wv_sb = wvp.tile([P, DK, GF], f32r, name="wv_sb")
            nc.sync.dma_start(
                out=wv_sb[:, :, :],
                in_=wv[:, :].rearrange("(kk p) f -> p kk f", p=P),
            )
            for tt in range(KT):
                xv = []
                for kk in range(DK):
                    t = xvp.tile([P, P], f32r, name="xv_t", tag="xv")
                    nc.sync.dma_start(
                        out=t[:, :],
                        in_=xT[kk * P:(kk + 1) * P, tt * P:(tt + 1) * P],
                    )
                    xv.append(t)
                vps = psum.tile([P, GF], f32, name="vps", tag="a", bufs=2)
                for kk in range(DK):
                    nc.tensor.matmul(
                        vps[:, :],
                        xv[kk][:, :],
                        wv_sb[:, kk, :],
                        start=(kk == 0), stop=(kk == DK - 1),
                    )
                nc.scalar.copy(
                    out=v_sb[tt][:, :].rearrange("p (h w) -> p h w", w=HD + 1)[:, :, 0:HD],
                    in_=vps[:, :].rearrange("p (h w) -> p h w", w=HD),
                )
                nc.vector.tensor_copy(
                    out=v_sb[tt][:, :].rearrange("p (h w) -> p h w", w=HD + 1)[:, :, HD:HD + 1],
                    in_=on_sb[:, 0:HPC].unsqueeze(2),
                )

        # ---- persistent weights for QK projection and output projection
        wqk_p = ctx.enter_context(tc.tile_pool(name="wqk", bufs=1))
        w_m = []
        for m in range(MT):
            t = wqk_p.tile([P, DK, P], f32r, name=f"wq_m{m}")
            nc.sync.dma_start(
                out=t[:, :, :],
                in_=wq[:, m * P:(m + 1) * P].rearrange("(kk p) f -> p kk f", p=P),
            )
            w_m.append(t)
        for m in range(MT):
            t = wqk_p.tile([P, DK, P], f32r, name=f"wk_m{m}")
            nc.sync.dma_start(
                out=t[:, :, :],
                in_=wk[:, m * P:(m + 1) * P].rearrange("(kk p) f -> p kk f", p=P),
            )
            w_m.append(t)
        wo_sb = [wqk_p.tile([P, D], f32r, name=f"wo_sb{kk}") for kk in range(MT)]
        for kk in range(MT):
            nc.sync.dma_start(out=wo_sb[kk][:, :], in_=wo[kk * P:(kk + 1) * P, :])

        xc_p = ctx.enter_context(tc.tile_pool(name="xc", bufs=2 * DK))

        # =========== Fused per-chunk pipeline ===========
        for j in range(NT):
            jc = slice(j * NCHUNK, (j + 1) * NCHUNK)

            # ---- A2: project q and k for chunk j
            xc = []
            for kk in range(DK):
                t = xc_p.tile([P, NCHUNK], f32r, name="xc_t", tag="xc")
                nc.sync.dma_start(out=t[:, :], in_=xT[kk * P:(kk + 1) * P, jc])
                xc.append(t)
            q_c = []
            for m in range(2 * MT):
                qps = psum.tile([P, NCHUNK], f32, name="qps", tag="a", bufs=2)
                for kk in range(DK):
                    nc.tensor.matmul(
                        qps[:, :], w_m[m][:, kk, :], xc[kk][:, :],
                        start=(kk == 0), stop=(kk == DK - 1),
                    )
                if m < MT:
                    qt = sb.tile([P, NCHUNK], bf16, name="q_c", tag=f"qc{m}", bufs=2)
                    nc.vector.tensor_copy(out=qt[:, :], in_=qps[:, :])
                    q_c.append(qt)
                else:
                    nc.vector.tensor_copy(out=k_sb[m - MT][:, jc], in_=qps[:, :])

            # ---- B: attention for chunk j, all head pairs
            a_c = [sb.tile([P, NCHUNK], f32r, name="a_c", tag=f"ac{m}", bufs=2)
                   for m in range(MT)]
            n_i = min(4 * j + 4, KT)
            for pr in range(NPAIR):
                av = [psum.tile([P, NCHUNK], f32, name="av_ps", tag="av", bufs=2)
                      for _ in range(2)]

                def av_mms(i, e2):
                    for hh in range(2):
                        nc.tensor.matmul(
                            av[hh][0:HD + 1, :],
                            v_sb[i][:, (2 * pr + hh) * (HD + 1):(2 * pr + hh + 1) * (HD + 1)],
                            e2[:, hh * NCHUNK:(hh + 1) * NCHUNK],
                            start=(i == 0), stop=(i == n_i - 1),
                        )

                prev = None
                for i in range(n_i):
                    d = i - 4 * j
                    s2 = psum.tile([P, 2 * NCHUNK], f32, name="s2", tag="s", bufs=2)
                    for hh in range(2):
                        hp = hh * HD
                        nc.tensor.matmul(
                            s2[:, hh * NCHUNK:(hh + 1) * NCHUNK],
                            k_sb[pr][hp:hp + HD, i * P:(i + 1) * P],
                            q_c[pr][hp:hp + HD, :],
                            start=True, stop=True,
                        )
                    e2 = sb.tile([P, 2 * NCHUNK], f32r, name="e2", tag="e", bufs=3)
                    nc.scalar.activation(out=e2[:, :], in_=s2[:, :], func=Exp,
                                         scale=SCALE)
                    if d >= 0:
                        lo = P * d
                        for hh in range(2):
                            nc.vector.tensor_tensor(
                                out=e2[:, hh * NCHUNK:hh * NCHUNK + lo + P],
                                in0=e2[:, hh * NCHUNK:hh * NCHUNK + lo + P],
                                in1=mk_sb[:, 3 * P - lo:4 * P], op=mult,
                            )
                    if prev is not None:
                        av_mms(*prev)
                    prev = (i, e2)
                av_mms(*prev)

                for hh in range(2):
                    nt = sb.tile([P, NCHUNK], f32r, name="nt", tag="nt", bufs=2)
                    nc.vector.tensor_copy(out=nt[0:HD + 1, :], in_=av[hh][0:HD + 1, :])
                    rt = sb.tile([P, NCHUNK], f32r, name="rt", tag="rt", bufs=2)
                    with nc.allow_low_precision(reason="FP22 softmax denominators"):
                        nc.vector.reciprocal(out=rt[64:65, :], in_=nt[64:65, :])
                    d_ps = psum.tile([HD, NCHUNK], f32, name="d_ps", tag="a", bufs=2)
                    nc.tensor.matmul(d_ps[:, :], ones_sb[64:65, :], rt[64:65, :],
                                     start=True, stop=True)
                    if hh == 0:
                        nc.vector.tensor_tensor(
                            out=a_c[pr][0:HD, :], in0=nt[0:HD, :], in1=d_ps[:, :],
                            op=mult,
                        )
                    else:
                        tmp = sb.tile([HD, NCHUNK], f32r, name="tmp", tag="tmp", bufs=2)
                        nc.vector.tensor_tensor(
                            out=tmp[:, :], in0=nt[0:HD, :], in1=d_ps[:, :], op=mult,
                        )
                        nc.sync.dma_start(out=a_c[pr][HD:2 * HD, :], in_=tmp[:, :])

            # ---- C: output projection for chunk j
            for m in range(D // P):
                ops = psum.tile([P, NCHUNK], f32, name="ops", tag="a", bufs=2)
                for kk in range(MT):
                    nc.tensor.matmul(
                        ops[:, :], wo_sb[kk][:, m * P:(m + 1) * P], a_c[kk][:, :],
                        start=(kk == 0), stop=(kk == MT - 1),
                    )
                oc = sb.tile([P, NCHUNK], f32, name="oc", tag="oc", bufs=2)
                nc.vector.tensor_copy(out=oc[:, :], in_=ops[:, :])
                nc.sync.dma_start(out=outT[m * P:(m + 1) * P, jc], in_=oc[:, :])

    nc.compile()
    return nc


def _shard_inputs(x, Wqkv, Wo):
    in_maps = []
    for c in range(8):
        b, g = c // 2, c % 2
        lo, hi = GF * g, GF * (g + 1)
        in_maps.append({
            "xT": np.ascontiguousarray(x[b].T),
            "wq": np.ascontiguousarray(Wqkv[:, lo:hi]),
            "wk": np.ascontiguousarray(Wqkv[:, D + lo:D + hi]),
            "wv": np.ascontiguousarray(Wqkv[:, 2 * D + lo:2 * D + hi]),
            "wo": np.ascontiguousarray(Wo[lo:hi, :]),
        })
    return in_maps


def kernel(x, Wqkv, Wo, _trace=False):
    from concourse.bass_utils import run_bass_kernel_spmd

    x = np.asarray(x, dtype=np.float32)
    Wqkv = np.asarray(Wqkv, dtype=np.float32)
    Wo = np.asarray(Wo, dtype=np.float32)
    B, T, _ = x.shape

    key = ("nc", T)
    if key not in _CACHE:
        _CACHE[key] = _build(T)
    nc = _CACHE[key]

    res = run_bass_kernel_spmd(nc, _shard_inputs(x, Wqkv, Wo), list(range(8)),
                               trace=_trace)
    out = np.empty((B, T, D), dtype=np.float32)
    for b in range(B):
        out[b] = (res.results[2 * b]["outT"] + res.results[2 * b + 1]["outT"]).T
    if _trace:
        kernel.last_results = res
    return out
